# revision 1
# baseline (speedup 1.0000x reference)
"""KuraNet Trainium2 kernel.

Pipeline (8 NeuronCores, SPMD, core c owns pair-rows i in [128c, 128c+128)):
  - L1 of the pair-MLP is separable: h1[(i,j),f] = x_i.W1A_f + x_j.W1B_f, so it
    reduces to two tiny [1024,16]@[16,128] matmuls (u, v).
  - BN1 batch stats over the N^2 Cartesian pair grid are exact in closed form
    from the 16x16 covariance of x (cross-covariance over the product grid
    vanishes), so no pass over N^2 is needed for BN1.
  - Pass 1 over the core's 131072 pairs: g = Lrelu(a1*(u_i+v_j)+c1) on ACT,
    h2 = g @ W2 on PE, bn_stats on DVE -> local BN2 moments.
  - One AllGather of per-core (sum, sumsq) -> exact global BN2 stats.
  - Pass 2: rebuild g, h2 = g@W2, g2 = Lrelu(a2*h2+c2), k = g2^T @ w3 (g2 as
    the stationary operand so k lands partition-major) -> transposed k block.
  - One AllGather of k blocks; softmax (max, exp, global sum) computed
    redundantly per core on the full logits; K = softmax * N;
    Ksym = .5(K + K^T) via 64 PE transposes. Every core holds full Ksym.
  - 150 explicit-Euler Kuramoto steps, replicated per core (no per-step
    collectives): dtheta = (cos th*(K@sin th) - sin th*(K@cos th))/N; K@[s|c]
    as 64 accumulating [128,128]x[128,2] matmuls with Ksym blocks stationary
    (K symmetric so block (jb,ib) is exactly the needed transpose).
  - sin/cos via ACT Sin after a DVE range-wrap into [-pi, pi].
Output traj is produced partition-major [128, 150, 8] and unsharded on host.
"""

import math

import numpy as np

import concourse.bass as bass
import concourse.bacc as bacc
import concourse.tile as tile
import concourse.mybir as mybir
from concourse.bass_utils import run_bass_kernel_spmd

N = 1024
FD = 16
H = 128
P = 128
NB = 8
NCORES = 8
STEPS = 150
ALPHA = 0.1
EPS = 1e-5
SLOPE = 0.01
PI = math.pi
F32 = mybir.dt.float32
AF = mybir.ActivationFunctionType
OP = mybir.AluOpType
AX = mybir.AxisListType


def build_program(steps=STEPS, debug=False):
    nc = bacc.Bacc("TRN2", target_bir_lowering=False, debug=False,
                   num_devices=NCORES)
    ins = {}
    for name, sh in [("xT", [FD, N]), ("x8", [NB, P, FD]), ("xbT", [FD, P]),
                     ("w1a", [FD, H]), ("w1b", [FD, H]), ("w2", [H, H]),
                     ("w3l", [H, 1]), ("b1", [H, 1]), ("g1", [H, 1]),
                     ("be1", [H, 1]), ("g2", [H, 1]), ("be2", [H, 1]),
                     ("ident", [P, P]), ("theta0", [P, NB])]:
        ins[name] = nc.dram_tensor(name, sh, F32, kind="ExternalInput")
    traj_ext = nc.dram_tensor("traj_pb", [P, steps, NB], F32,
                              kind="ExternalOutput")
    ksym_ext = (nc.dram_tensor("ksym_dbg", [P, NB, N], F32,
                               kind="ExternalOutput") if debug else None)

    stats_in = nc.dram_tensor("stats_in", [H, 2], F32)
    stats_sh = nc.dram_tensor("stats_sh", [NCORES, H, 2], F32,
                              addr_space="Shared")
    k_in = nc.dram_tensor("k_in", [P, N], F32)
    k_sh = nc.dram_tensor("k_sh", [NCORES, P, N], F32, addr_space="Shared")
    rg = [list(range(NCORES))]

    from contextlib import ExitStack
    with tile.TileContext(nc) as tc, ExitStack() as ctx:
        const = ctx.enter_context(tc.tile_pool(name="const", bufs=1))
        big = ctx.enter_context(tc.tile_pool(name="big", bufs=1))
        work = ctx.enter_context(tc.tile_pool(name="work", bufs=3))
        g2p = ctx.enter_context(tc.tile_pool(name="g2p", bufs=2))
        small = ctx.enter_context(tc.tile_pool(name="small", bufs=1))
        ps = ctx.enter_context(tc.tile_pool(name="ps", bufs=2, space="PSUM"))
        psk = ctx.enter_context(tc.tile_pool(name="psk", bufs=1, space="PSUM"))
        pso = ctx.enter_context(tc.tile_pool(name="pso", bufs=1, space="PSUM"))

        def load(name, sh):
            t = const.tile(sh, F32, tag=name)
            nc.sync.dma_start(out=t[:], in_=ins[name][:])
            return t

        sXT = load("xT", [FD, N])
        sXBT = load("xbT", [FD, P])
        sW1A = load("w1a", [FD, H])
        sW1B = load("w1b", [FD, H])
        sW2 = load("w2", [H, H])
        sW3 = load("w3l", [H, 1])
        sB1 = load("b1", [H, 1])
        sG1 = load("g1", [H, 1])
        sBE1 = load("be1", [H, 1])
        sG2 = load("g2", [H, 1])
        sBE2 = load("be2", [H, 1])
        sID = load("ident", [P, P])
        sX8 = const.tile([P, NB, FD], F32, tag="x8")
        nc.sync.dma_start(out=sX8[:], in_=ins["x8"][:].rearrange("b p k -> p b k"))

        # ---- BN1 closed-form setup ----
        xb = small.tile([FD, 1], F32)
        nc.vector.tensor_reduce(out=xb[:], in_=sXT[:], axis=AX.X, op=OP.add)
        nc.vector.tensor_scalar_mul(xb[:], xb[:], 1.0 / N)

        mA = small.tile([H, 1], F32)
        mB = small.tile([H, 1], F32)
        for w, m in ((sW1A, mA), (sW1B, mB)):
            pm = ps.tile([H, 1], F32, tag="setup")
            nc.tensor.matmul(pm[:], w[:], xb[:], start=True, stop=True)
            nc.vector.tensor_copy(m[:], pm[:])
        m1 = small.tile([H, 1], F32)   # mu1 + b1
        nc.vector.tensor_add(m1[:], mA[:], mB[:])
        nc.vector.tensor_add(m1[:], m1[:], sB1[:])

        pS = ps.tile([FD, FD], F32, tag="setup")
        for b in range(NB):
            nc.tensor.matmul(pS[:], sX8[:, b, :], sX8[:, b, :],
                             start=(b == 0), stop=(b == NB - 1))
        sS = small.tile([FD, FD], F32)
        nc.vector.tensor_copy(sS[:], pS[:])
        ones = small.tile([P, 1], F32)
        nc.vector.memset(ones[:], 1.0)
        pqs = ps.tile([1, H], F32, tag="setup")
        for half, w in enumerate((sW1A, sW1B)):
            pSA = ps.tile([FD, H], F32, tag="setup")
            nc.tensor.matmul(pSA[:], sS[:], w[:], start=True, stop=True)
            qa = small.tile([FD, H], F32, tag=f"qa{half}")
            nc.vector.tensor_mul(qa[:], pSA[:], w[:])
            nc.tensor.matmul(pqs[:], ones[0:FD, :], qa[:],
                             start=(half == 0), stop=(half == 1))
        qsum = small.tile([1, H], F32)
        nc.vector.tensor_copy(qsum[:], pqs[:])
        pq = ps.tile([H, 1], F32, tag="setup")
        nc.tensor.transpose(pq[:], qsum[:], sID[0:1, 0:1])
        t1 = small.tile([H, 1], F32, tag="t1")
        nc.vector.tensor_mul(t1[:], mA[:], mA[:])
        var1 = small.tile([H, 1], F32)
        nc.vector.scalar_tensor_tensor(out=var1[:], in0=pq[:], scalar=1.0 / N,
                                       in1=t1[:], op0=OP.mult, op1=OP.subtract)
        nc.vector.tensor_mul(t1[:], mB[:], mB[:])
        nc.vector.tensor_sub(var1[:], var1[:], t1[:])
        eps_t = small.tile([H, 1], F32)
        nc.vector.memset(eps_t[:], EPS)
        sd = small.tile([H, 1], F32)
        nc.scalar.activation(out=sd[:], in_=var1[:], func=AF.Sqrt, bias=eps_t[:])
        a1 = small.tile([H, 1], F32)
        nc.vector.reciprocal(a1[:], sd[:])
        nc.vector.tensor_mul(a1[:], a1[:], sG1[:])
        c1 = small.tile([H, 1], F32)
        nc.vector.tensor_mul(c1[:], a1[:], m1[:])
        nc.vector.tensor_sub(c1[:], sBE1[:], c1[:])

        pu = ps.tile([H, P], F32, tag="setup")
        nc.tensor.matmul(pu[:], sW1A[:], sXBT[:], start=True, stop=True)
        su = const.tile([H, P], F32, tag="su")
        nc.scalar.activation(out=su[:], in_=pu[:], func=AF.Identity,
                             bias=c1[:], scale=a1[:])
        av = const.tile([H, N], F32, tag="av")
        for h in range(2):
            pv = ps.tile([H, 512], F32, tag="setup")
            nc.tensor.matmul(pv[:], sW1B[:], sXT[:, h * 512:(h + 1) * 512],
                             start=True, stop=True)
            nc.scalar.activation(out=av[:, h * 512:(h + 1) * 512], in_=pv[:],
                                 func=AF.Identity, scale=a1[:])

        # ---- pass 1: BN2 moments ----
        stats = big.tile([P, P, 2, 6], F32, tag="stats")
        for i in range(P):
            g = work.tile([H, N], F32, tag="g")
            nc.scalar.activation(out=g[:], in_=av[:], func=AF.Lrelu,
                                 bias=su[:, i:i + 1], alpha=SLOPE)
            for h in range(2):
                ph2 = ps.tile([H, 512], F32, tag="ph2")
                nc.tensor.matmul(ph2[:], sW2[:], g[:, h * 512:(h + 1) * 512],
                                 start=True, stop=True)
                nc.vector.bn_stats(out=stats[:, i, h, :], in_=ph2[:])
        mv = small.tile([H, 2], F32)
        nc.vector.bn_aggr(out=mv[:], in_=stats[:].rearrange("p i h s -> p (i h) s"))
        CNT = float(P * N)
        ex = small.tile([H, 2], F32)
        nc.vector.tensor_scalar_mul(ex[:, 0:1], mv[:, 0:1], CNT)
        tq = small.tile([H, 1], F32, tag="tq")
        nc.vector.tensor_mul(tq[:], mv[:, 0:1], mv[:, 0:1])
        nc.vector.tensor_add(tq[:], tq[:], mv[:, 1:2])
        nc.vector.tensor_scalar_mul(ex[:, 1:2], tq[:], CNT)
        nc.sync.dma_start(out=stats_in[:], in_=ex[:])
        nc.gpsimd.collective_compute("AllGather", OP.bypass, replica_groups=rg,
                                     ins=[stats_in[:]], outs=[stats_sh[:]])
        sg = small.tile([H, NCORES, 2], F32)
        nc.sync.dma_start(out=sg[:], in_=stats_sh[:].rearrange("r p s -> p r s"))
        tot = small.tile([H, 2], F32)
        nc.vector.tensor_reduce(out=tot[:, 0:1], in_=sg[:, :, 0], axis=AX.X,
                                op=OP.add)
        nc.vector.tensor_reduce(out=tot[:, 1:2], in_=sg[:, :, 1], axis=AX.X,
                                op=OP.add)
        TOT = float(NCORES * P * N)
        mean2 = small.tile([H, 1], F32)
        nc.vector.tensor_scalar_mul(mean2[:], tot[:, 0:1], 1.0 / TOT)
        var2 = small.tile([H, 1], F32)
        nc.vector.tensor_scalar_mul(var2[:], tot[:, 1:2], 1.0 / TOT)
        tm = small.tile([H, 1], F32, tag="tm")
        nc.vector.tensor_mul(tm[:], mean2[:], mean2[:])
        nc.vector.tensor_sub(var2[:], var2[:], tm[:])
        sd2 = small.tile([H, 1], F32)
        nc.scalar.activation(out=sd2[:], in_=var2[:], func=AF.Sqrt, bias=eps_t[:])
        a2 = small.tile([H, 1], F32)
        nc.vector.reciprocal(a2[:], sd2[:])
        nc.vector.tensor_mul(a2[:], a2[:], sG2[:])
        c2 = small.tile([H, 1], F32)
        nc.vector.tensor_mul(c2[:], a2[:], mean2[:])
        nc.vector.tensor_sub(c2[:], sBE2[:], c2[:])

        # ---- pass 2: k logits (transposed block layout) ----
        pkb0 = psk.tile([P, NB, 64], F32, tag="pk0")
        pkb1 = psk.tile([P, NB, 64], F32, tag="pk1")
        pkb = [pkb0, pkb1]
        for i in range(P):
            g = work.tile([H, N], F32, tag="g")
            nc.scalar.activation(out=g[:], in_=av[:], func=AF.Lrelu,
                                 bias=su[:, i:i + 1], alpha=SLOPE)
            g2t = g2p.tile([H, N], F32, tag="g2")
            if i % 4 == 3:
                for h in range(2):
                    ph2 = ps.tile([H, 512], F32, tag="ph2")
                    nc.tensor.matmul(ph2[:], sW2[:], g[:, h * 512:(h + 1) * 512],
                                     start=True, stop=True)
                    nc.scalar.activation(out=g2t[:, h * 512:(h + 1) * 512],
                                         in_=ph2[:], func=AF.Lrelu,
                                         bias=c2[:], scale=a2[:], alpha=SLOPE)
            else:
                zt = g2p.tile([H, N], F32, tag="z")
                for h in range(2):
                    ph2 = ps.tile([H, 512], F32, tag="ph2")
                    nc.tensor.matmul(ph2[:], sW2[:], g[:, h * 512:(h + 1) * 512],
                                     start=True, stop=True)
                    nc.vector.tensor_scalar(out=zt[:, h * 512:(h + 1) * 512],
                                            in0=ph2[:], scalar1=a2[:],
                                            scalar2=c2[:], op0=OP.mult,
                                            op1=OP.add)
                nc.vector.scalar_tensor_tensor(out=g2t[:], in0=zt[:],
                                               scalar=SLOPE, in1=zt[:],
                                               op0=OP.mult, op1=OP.max)
            bank, slot = divmod(i, 64)
            for jb in range(NB):
                nc.tensor.matmul(pkb[bank][:, jb, slot:slot + 1],
                                 g2t[:, jb * P:(jb + 1) * P], sW3[:],
                                 start=True, stop=True)
        KT = big.tile([P, NB, P], F32, tag="KT")
        for bank in range(2):
            nc.vector.tensor_copy(KT[:, :, bank * 64:(bank + 1) * 64],
                                  pkb[bank][:])
        nc.sync.dma_start(out=k_in[:], in_=KT[:].rearrange("p j f -> p (j f)"))
        nc.gpsimd.collective_compute("AllGather", OP.bypass, replica_groups=rg,
                                     ins=[k_in[:]], outs=[k_sh[:]])
        # kallT[p, r, s, f] = k(128r+f, 128s+p)
        kallT = big.tile([P, NB, NB, P], F32, tag="kallT")
        nc.sync.dma_start(out=kallT[:],
                          in_=k_sh[:].rearrange("r p (s f) -> p r s f", s=NB))

        # ---- softmax * N and symmetrize (replicated) ----
        rm = small.tile([P, 1], F32)
        nc.vector.tensor_reduce(out=rm[:],
                                in_=kallT[:].rearrange("p r s f -> p (r s f)"),
                                axis=AX.X, op=OP.max)
        prm = ps.tile([1, P], F32, tag="setup")
        nc.tensor.transpose(prm[:], rm[:], sID[:])
        gm = small.tile([1, 1], F32)
        nc.vector.tensor_reduce(out=gm[:], in_=prm[:], axis=AX.X, op=OP.max)
        nc.vector.tensor_scalar_mul(gm[:], gm[:], -1.0)
        nM = small.tile([P, 1], F32)
        nc.gpsimd.partition_broadcast(nM[:], gm[:])
        ET = big.tile([P, NB, NB, P], F32, tag="ET")
        es = small.tile([P, NB], F32)
        for r in range(NB):
            nc.scalar.activation(out=ET[:, r, :, :], in_=kallT[:, r, :, :],
                                 func=AF.Exp, bias=nM[:],
                                 accum_out=es[:, r:r + 1])
        rs = small.tile([P, 1], F32)
        nc.vector.tensor_reduce(out=rs[:], in_=es[:], axis=AX.X, op=OP.add)
        pz = ps.tile([1, 1], F32, tag="setup")
        nc.tensor.matmul(pz[:], ones[:], rs[:], start=True, stop=True)
        z1 = small.tile([1, 1], F32)
        nc.vector.reciprocal(z1[:], pz[:])
        nc.vector.tensor_scalar_mul(z1[:], z1[:], 0.5 * N)
        sc = small.tile([P, 1], F32)
        nc.gpsimd.partition_broadcast(sc[:], z1[:])
        for r in range(NB):
            nc.vector.tensor_scalar_mul(ET[:, r, :, :], ET[:, r, :, :], sc[:])
        # KS[:, a, b, :] = Ksym_blk(a,b) = T(ET[:, a, b, :]) + ET[:, b, a, :]
        BF16 = mybir.dt.bfloat16
        KSb = big.tile([P, NB, NB, P], BF16, tag="KSb")
        KS = None
        if debug:
            KS = big.tile([P, NB, NB, P], F32, tag="KS")
        for a in range(NB):
            for b in range(NB):
                pt = pso.tile([P, P], F32, tag="pt")
                nc.tensor.transpose(pt[:], ET[:, a, b, :], sID[:])
                nc.vector.tensor_add(KSb[:, a, b, :], ET[:, b, a, :], pt[:])
                if debug:
                    nc.vector.tensor_add(KS[:, a, b, :], ET[:, b, a, :], pt[:])
        if debug:
            nc.sync.dma_start(out=ksym_ext[:],
                              in_=KS[:].rearrange("p a b f -> p a (b f)"))

        # ---- ODE: explicit Euler, fully replicated ----
        traj = big.tile([P, steps, NB], F32, tag="traj")
        th0 = small.tile([P, NB], F32)
        nc.sync.dma_start(out=th0[:], in_=ins["theta0"][:])
        for t in range(steps):
            prev = th0[:] if t == 0 else traj[:, t - 1, :]
            wb = work.tile([P, 16], F32, tag="wb")
            nc.vector.add_range_wrap(out=wb[:, 0:8], in_=prev, shift=0.0,
                                     bound=PI, period=2 * PI)
            nc.vector.add_range_wrap(out=wb[:, 8:16], in_=prev, shift=PI / 2,
                                     bound=PI, period=2 * PI)
            sct = work.tile([P, NB, 2], F32, tag="sct")
            nc.scalar.activation(out=sct[:].rearrange("p a b -> p b a"),
                                 in_=wb[:], func=AF.Sin)
            sctb = work.tile([P, NB, 2], BF16, tag="sctb")
            nc.vector.tensor_copy(sctb[:], sct[:])
            po = pso.tile([P, NB, 2], F32, tag="po")
            for ib in range(NB):
                for jb in range(NB):
                    nc.tensor.matmul(po[:, ib, :], KSb[:, jb, ib, :],
                                     sctb[:, jb, :], start=(jb == 0),
                                     stop=(jb == NB - 1))
            d1 = work.tile([P, NB], F32, tag="d1")
            nc.vector.tensor_mul(d1[:], sct[:, :, 1], po[:, :, 0])
            d2 = work.tile([P, NB], F32, tag="d2")
            nc.vector.tensor_mul(d2[:], sct[:, :, 0], po[:, :, 1])
            nc.vector.tensor_sub(d1[:], d1[:], d2[:])
            nc.vector.scalar_tensor_tensor(out=traj[:, t, :], in0=d1[:],
                                           scalar=ALPHA / N, in1=prev,
                                           op0=OP.mult, op1=OP.add)
        nc.sync.dma_start(out=traj_ext[:], in_=traj[:])

    nc.compile()
    return nc


_CACHED = {}


def _get_program(steps=STEPS, debug=False):
    key = (steps, debug)
    if key not in _CACHED:
        _CACHED[key] = build_program(steps, debug)
    return _CACHED[key]


def make_in_maps(inputs, theta0=None):
    x = np.ascontiguousarray(np.asarray(inputs["x"], dtype=np.float32))
    w1 = np.asarray(inputs["w1"], np.float32)
    if theta0 is None:
        th0 = np.zeros((P, NB), np.float32)
    else:
        th0 = np.ascontiguousarray(
            np.asarray(theta0, np.float32).reshape(NB, P).T)
    base = {
        "xT": np.ascontiguousarray(x.T),
        "x8": np.ascontiguousarray(x.reshape(NB, P, FD)),
        "w1a": np.ascontiguousarray(w1[:FD]),
        "w1b": np.ascontiguousarray(w1[FD:]),
        "w2": np.asarray(inputs["w2"], np.float32),
        "w3l": np.asarray(inputs["w3"], np.float32).reshape(H, 1),
        "b1": np.asarray(inputs["b1"], np.float32).reshape(H, 1),
        "g1": np.asarray(inputs["gamma1"], np.float32).reshape(H, 1),
        "be1": np.asarray(inputs["beta1"], np.float32).reshape(H, 1),
        "g2": np.asarray(inputs["gamma2"], np.float32).reshape(H, 1),
        "be2": np.asarray(inputs["beta2"], np.float32).reshape(H, 1),
        "ident": np.eye(P, dtype=np.float32),
        "theta0": th0,
    }
    maps = []
    for c in range(NCORES):
        m = dict(base)
        m["xbT"] = np.ascontiguousarray(x[c * P:(c + 1) * P].T)
        maps.append(m)
    return maps


def unpack_traj(traj_pb, steps):
    return np.ascontiguousarray(
        traj_pb.transpose(1, 2, 0).reshape(steps, N).astype(np.float32))


def unpack_ksym(ksym_dbg):
    return np.ascontiguousarray(
        ksym_dbg.reshape(P, NB, N).transpose(1, 0, 2).reshape(N, N))


def run(inputs, steps=STEPS, theta0=None, debug=True):
    nc = _get_program(steps, debug)
    res = run_bass_kernel_spmd(nc, make_in_maps(inputs, theta0),
                               list(range(NCORES)))
    return res.results


def kernel(**inputs):
    results = run(inputs, debug=False)
    return unpack_traj(results[0]["traj_pb"], STEPS)



# revision 6
# speedup vs baseline: 1151.4569x; 1151.4569x over previous
"""KuraNet Trainium2 kernel.

Pipeline (8 NeuronCores, SPMD, core c owns pair-rows i in [128c, 128c+128)):
  - L1 of the pair-MLP is separable: h1[(i,j),f] = x_i.W1A_f + x_j.W1B_f, so it
    reduces to two tiny [1024,16]@[16,128] matmuls (u, v).
  - BN1 batch stats over the N^2 Cartesian pair grid are exact in closed form
    from the 16x16 covariance of x (cross-covariance over the product grid
    vanishes), so no pass over N^2 is needed for BN1.
  - Pass 1 over the core's 131072 pairs: g = Lrelu(a1*(u_i+v_j)+c1) on ACT in
    bf16, h2 = g @ W2 on PE (bf16 operands, fp32 PSUM), one bn_stats per row
    on DVE over the [H, 1024] two-bank PSUM tile -> local BN2 moments.
  - One AllGather of per-core (sum, sumsq) -> exact global BN2 stats.
  - Pass 2: rebuild g (ACT), h2 = g@W2 (PE bf16), z2 = a2*h2+c2 (DVE,
    PSUM->bf16), g2 = Lrelu(z2) (GpSimd, SBUF->SBUF), k = g2^T @ w3 with g2
    blocks stationary (bf16 -> FWL weight loads) -> transposed k block.
  - One AllGather of k blocks; softmax computed redundantly per core with NO
    max-subtraction (logits are O(+-10), f32 exp + f32 accumulate is safe):
    one big ACT Exp over [P, 8192] -> bf16 E + f32 row-sums; the softmax
    scale (N/Z), the 0.5 of the symmetrization and the Euler alpha/N all fold
    into one per-step scalar, so E is never rescaled.
  - Ksym_raw = E + E^T via 64 bf16 PE transposes + DVE adds. Every core holds
    the full unscaled symmetric coupling matrix in bf16.
  - 150 explicit-Euler Kuramoto steps, replicated per core (no per-step
    collectives): wrap theta three ways (0, +pi/2, +pi), one ACT Sin gives
    (sin, cos, -sin) in bf16; 64 accumulating [128,128]x[128,2] matmuls with
    Ksym_raw blocks stationary (K symmetric so block (jb,ib) is the needed
    transpose); dtheta = sum_c (K@[s|c]) * [cos|-sin] via one DVE mult and an
    innermost-axis reduce; theta += (0.5*alpha/Z) * dtheta.
Output traj is produced partition-major [128, 150, 8] and unsharded on host.
"""

import math

import numpy as np

import concourse.bass as bass
import concourse.bacc as bacc
import concourse.tile as tile
import concourse.mybir as mybir
from concourse.bass_utils import run_bass_kernel_spmd

N = 1024
FD = 16
H = 128
P = 128
NB = 8
NCORES = 8
STEPS = 150
ALPHA = 0.1
EPS = 1e-5
SLOPE = 0.01
PI = math.pi
F32 = mybir.dt.float32
BF16 = mybir.dt.bfloat16
AF = mybir.ActivationFunctionType
OP = mybir.AluOpType
AX = mybir.AxisListType


def build_program(steps=STEPS, debug=False):
    nc = bacc.Bacc("TRN2", target_bir_lowering=False, debug=False,
                   num_devices=NCORES)
    ins = {}
    for name, sh in [("xT", [FD, N]), ("x8", [NB, P, FD]), ("xbT", [FD, P]),
                     ("w1a", [FD, H]), ("w1b", [FD, H]), ("w2", [H, H]),
                     ("w3l", [H, 1]), ("b1", [H, 1]), ("g1", [H, 1]),
                     ("be1", [H, 1]), ("g2", [H, 1]), ("be2", [H, 1]),
                     ("ident", [P, P]), ("theta0", [P, NB])]:
        ins[name] = nc.dram_tensor(name, sh, F32, kind="ExternalInput")
    traj_ext = nc.dram_tensor("traj_pb", [P, steps, NB], F32,
                              kind="ExternalOutput")
    ksym_ext = (nc.dram_tensor("ksym_dbg", [P, NB, N], F32,
                               kind="ExternalOutput") if debug else None)

    stats_in = nc.dram_tensor("stats_in", [H, 2], F32)
    stats_sh = nc.dram_tensor("stats_sh", [NCORES, H, 2], F32,
                              addr_space="Shared")
    k_in = nc.dram_tensor("k_in", [P, N], F32)
    k_sh = nc.dram_tensor("k_sh", [NCORES, P, N], F32, addr_space="Shared")
    rg = [list(range(NCORES))]

    from contextlib import ExitStack
    with tile.TileContext(nc) as tc, ExitStack() as ctx:
        const = ctx.enter_context(tc.tile_pool(name="const", bufs=1))
        big = ctx.enter_context(tc.tile_pool(name="big", bufs=1))
        work = ctx.enter_context(tc.tile_pool(name="work", bufs=3))
        g2p = ctx.enter_context(tc.tile_pool(name="g2p", bufs=2))
        small = ctx.enter_context(tc.tile_pool(name="small", bufs=1))
        ps = ctx.enter_context(tc.tile_pool(name="ps", bufs=1, space="PSUM"))
        php = ctx.enter_context(tc.tile_pool(name="php", bufs=2, space="PSUM"))
        psk = ctx.enter_context(tc.tile_pool(name="psk", bufs=1, space="PSUM"))

        def load(name, sh):
            t = const.tile(sh, F32, tag=name)
            nc.sync.dma_start(out=t[:], in_=ins[name][:])
            return t

        sXT = load("xT", [FD, N])
        sXBT = load("xbT", [FD, P])
        sW1A = load("w1a", [FD, H])
        sW1B = load("w1b", [FD, H])
        sW2 = load("w2", [H, H])
        sW3 = load("w3l", [H, 1])
        sB1 = load("b1", [H, 1])
        sG1 = load("g1", [H, 1])
        sBE1 = load("be1", [H, 1])
        sG2 = load("g2", [H, 1])
        sBE2 = load("be2", [H, 1])
        sID = load("ident", [P, P])
        sX8 = const.tile([P, NB, FD], F32, tag="x8")
        nc.sync.dma_start(out=sX8[:], in_=ins["x8"][:].rearrange("b p k -> p b k"))

        # bf16 copies for fast PE streaming / FWL weight loads
        sW2B = const.tile([H, H], BF16, tag="w2b")
        nc.vector.tensor_copy(sW2B[:], sW2[:])
        sW3B = const.tile([H, 1], BF16, tag="w3b")
        nc.vector.tensor_copy(sW3B[:], sW3[:])
        sIDB = const.tile([P, P], BF16, tag="identb")
        nc.vector.tensor_copy(sIDB[:], sID[:])

        # ---- BN1 closed-form setup ----
        xb = small.tile([FD, 1], F32)
        nc.vector.tensor_reduce(out=xb[:], in_=sXT[:], axis=AX.X, op=OP.add)
        nc.vector.tensor_scalar_mul(xb[:], xb[:], 1.0 / N)

        mA = small.tile([H, 1], F32)
        mB = small.tile([H, 1], F32)
        for w, m in ((sW1A, mA), (sW1B, mB)):
            pm = ps.tile([H, 1], F32, tag="setup")
            nc.tensor.matmul(pm[:], w[:], xb[:], start=True, stop=True)
            nc.vector.tensor_copy(m[:], pm[:])
        m1 = small.tile([H, 1], F32)   # mu1 + b1
        nc.vector.tensor_add(m1[:], mA[:], mB[:])
        nc.vector.tensor_add(m1[:], m1[:], sB1[:])

        pS = ps.tile([FD, FD], F32, tag="setup")
        for b in range(NB):
            nc.tensor.matmul(pS[:], sX8[:, b, :], sX8[:, b, :],
                             start=(b == 0), stop=(b == NB - 1))
        sS = small.tile([FD, FD], F32)
        nc.vector.tensor_copy(sS[:], pS[:])
        ones = small.tile([P, 1], F32)
        nc.vector.memset(ones[:], 1.0)
        pqs = ps.tile([1, H], F32, tag="setup")
        for half, w in enumerate((sW1A, sW1B)):
            pSA = ps.tile([FD, H], F32, tag="setup2")
            nc.tensor.matmul(pSA[:], sS[:], w[:], start=True, stop=True)
            qa = small.tile([FD, H], F32, tag=f"qa{half}")
            nc.vector.tensor_mul(qa[:], pSA[:], w[:])
            nc.tensor.matmul(pqs[:], ones[0:FD, :], qa[:],
                             start=(half == 0), stop=(half == 1))
        qsum = small.tile([1, H], F32)
        nc.vector.tensor_copy(qsum[:], pqs[:])
        pq = ps.tile([H, 1], F32, tag="setup")
        nc.tensor.transpose(pq[:], qsum[:], sID[0:1, 0:1])
        t1 = small.tile([H, 1], F32, tag="t1")
        nc.vector.tensor_mul(t1[:], mA[:], mA[:])
        var1 = small.tile([H, 1], F32)
        nc.vector.scalar_tensor_tensor(out=var1[:], in0=pq[:], scalar=1.0 / N,
                                       in1=t1[:], op0=OP.mult, op1=OP.subtract)
        nc.vector.tensor_mul(t1[:], mB[:], mB[:])
        nc.vector.tensor_sub(var1[:], var1[:], t1[:])
        eps_t = small.tile([H, 1], F32)
        nc.vector.memset(eps_t[:], EPS)
        sd = small.tile([H, 1], F32)
        nc.scalar.activation(out=sd[:], in_=var1[:], func=AF.Sqrt, bias=eps_t[:])
        a1 = small.tile([H, 1], F32)
        nc.vector.reciprocal(a1[:], sd[:])
        nc.vector.tensor_mul(a1[:], a1[:], sG1[:])
        c1 = small.tile([H, 1], F32)
        nc.vector.tensor_mul(c1[:], a1[:], m1[:])
        nc.vector.tensor_sub(c1[:], sBE1[:], c1[:])

        pu = ps.tile([H, P], F32, tag="setup")
        nc.tensor.matmul(pu[:], sW1A[:], sXBT[:], start=True, stop=True)
        su = const.tile([H, P], F32, tag="su")
        nc.scalar.activation(out=su[:], in_=pu[:], func=AF.Identity,
                             bias=c1[:], scale=a1[:])
        av = const.tile([H, N], F32, tag="av")
        for h in range(2):
            pv = ps.tile([H, 512], F32, tag="setup")
            nc.tensor.matmul(pv[:], sW1B[:], sXT[:, h * 512:(h + 1) * 512],
                             start=True, stop=True)
            nc.scalar.activation(out=av[:, h * 512:(h + 1) * 512], in_=pv[:],
                                 func=AF.Identity, scale=a1[:])

        # ---- pass 1: BN2 moments ----
        stats = big.tile([P, P, 2, 6], F32, tag="stats")
        ph2_prev = None
        for i in range(P + 1):
            if ph2_prev is not None:
                for h in range(2):
                    nc.vector.bn_stats(out=stats[:, i - 1, h, :],
                                       in_=ph2_prev[:, h, :])
            if i == P:
                break
            g = work.tile([H, N], BF16, tag="g")
            nc.scalar.activation(out=g[:], in_=av[:], func=AF.Lrelu,
                                 bias=su[:, i:i + 1], alpha=SLOPE)
            ph2 = php.tile([H, 2, 512], F32, tag="ph2")
            for h in range(2):
                nc.tensor.matmul(ph2[:, h, :], sW2B[:],
                                 g[:, h * 512:(h + 1) * 512],
                                 start=True, stop=True)
            ph2_prev = ph2
        mv = small.tile([H, 2], F32)
        nc.vector.bn_aggr(out=mv[:],
                          in_=stats[:].rearrange("p i h s -> p (i h) s"))
        CNT = float(P * N)
        ex = small.tile([H, 2], F32)
        nc.vector.tensor_scalar_mul(ex[:, 0:1], mv[:, 0:1], CNT)
        tq = small.tile([H, 1], F32, tag="tq")
        nc.vector.tensor_mul(tq[:], mv[:, 0:1], mv[:, 0:1])
        nc.vector.tensor_add(tq[:], tq[:], mv[:, 1:2])
        nc.vector.tensor_scalar_mul(ex[:, 1:2], tq[:], CNT)
        nc.sync.dma_start(out=stats_in[:], in_=ex[:])
        nc.gpsimd.collective_compute("AllGather", OP.bypass, replica_groups=rg,
                                     ins=[stats_in[:]], outs=[stats_sh[:]])
        sg = small.tile([H, NCORES, 2], F32)
        nc.sync.dma_start(out=sg[:], in_=stats_sh[:].rearrange("r p s -> p r s"))
        tot = small.tile([H, 2], F32)
        nc.vector.tensor_reduce(out=tot[:, 0:1], in_=sg[:, :, 0], axis=AX.X,
                                op=OP.add)
        nc.vector.tensor_reduce(out=tot[:, 1:2], in_=sg[:, :, 1], axis=AX.X,
                                op=OP.add)
        TOT = float(NCORES * P * N)
        mean2 = small.tile([H, 1], F32)
        nc.vector.tensor_scalar_mul(mean2[:], tot[:, 0:1], 1.0 / TOT)
        var2 = small.tile([H, 1], F32)
        nc.vector.tensor_scalar_mul(var2[:], tot[:, 1:2], 1.0 / TOT)
        tm = small.tile([H, 1], F32, tag="tm")
        nc.vector.tensor_mul(tm[:], mean2[:], mean2[:])
        nc.vector.tensor_sub(var2[:], var2[:], tm[:])
        sd2 = small.tile([H, 1], F32)
        nc.scalar.activation(out=sd2[:], in_=var2[:], func=AF.Sqrt, bias=eps_t[:])
        a2 = small.tile([H, 1], F32)
        nc.vector.reciprocal(a2[:], sd2[:])
        nc.vector.tensor_mul(a2[:], a2[:], sG2[:])
        c2 = small.tile([H, 1], F32)
        nc.vector.tensor_mul(c2[:], a2[:], mean2[:])
        nc.vector.tensor_sub(c2[:], sBE2[:], c2[:])

        # ---- pass 2: k logits (transposed block layout) ----
        pkb0 = psk.tile([P, NB, 64], F32, tag="pk0")
        pkb1 = psk.tile([P, NB, 64], F32, tag="pk1")
        pkb = [pkb0, pkb1]
        QA = 416   # cols of g2 built fused on ACT; DVE takes the rest
        ph2_prev = None
        for i in range(P + 1):
            if ph2_prev is not None:
                j = i - 1
                g2t = g2p.tile([H, N], BF16, tag="g2")
                nc.scalar.activation(out=g2t[:, 0:QA],
                                     in_=ph2_prev[:, 0, 0:QA], func=AF.Lrelu,
                                     bias=c2[:], scale=a2[:], alpha=SLOPE)
                zt = work.tile([H, N], BF16, tag="zt")
                nc.vector.tensor_scalar(out=zt[:, QA:512],
                                        in0=ph2_prev[:, 0, QA:512],
                                        scalar1=a2[:], scalar2=c2[:],
                                        op0=OP.mult, op1=OP.add)
                nc.vector.tensor_scalar(out=zt[:, 512:1024],
                                        in0=ph2_prev[:, 1, :],
                                        scalar1=a2[:], scalar2=c2[:],
                                        op0=OP.mult, op1=OP.add)
                nc.vector.scalar_tensor_tensor(out=g2t[:, QA:], in0=zt[:, QA:],
                                               scalar=SLOPE, in1=zt[:, QA:],
                                               op0=OP.mult, op1=OP.max)
                bank, slot = divmod(j, 64)
                for jb in range(NB):
                    nc.tensor.matmul(pkb[bank][:, jb, slot:slot + 1],
                                     g2t[:, jb * P:(jb + 1) * P], sW3B[:],
                                     start=True, stop=True)
            if i == P:
                break
            g = work.tile([H, N], BF16, tag="g")
            nc.scalar.activation(out=g[:], in_=av[:], func=AF.Lrelu,
                                 bias=su[:, i:i + 1], alpha=SLOPE)
            ph2 = php.tile([H, 2, 512], F32, tag="ph2")
            for h in range(2):
                nc.tensor.matmul(ph2[:, h, :], sW2B[:],
                                 g[:, h * 512:(h + 1) * 512],
                                 start=True, stop=True)
            ph2_prev = ph2
        KT = big.tile([P, NB, P], F32, tag="KT")
        for bank in range(2):
            nc.vector.tensor_copy(KT[:, :, bank * 64:(bank + 1) * 64],
                                  pkb[bank][:])
        nc.sync.dma_start(out=k_in[:], in_=KT[:].rearrange("p j f -> p (j f)"))
        nc.gpsimd.collective_compute("AllGather", OP.bypass, replica_groups=rg,
                                     ins=[k_in[:]], outs=[k_sh[:]])
        # kallT[p, r, s, f] = k(128r+f, 128s+p)
        kallT = big.tile([P, NB, NB, P], F32, tag="kallT")
        nc.sync.dma_start(out=kallT[:],
                          in_=k_sh[:].rearrange("r p (s f) -> p r s f", s=NB))

        # ---- exp (no max-subtraction; scale folded into the ODE step) ----
        ET = big.tile([P, NB, NB, P], BF16, tag="ET")
        rs = small.tile([P, 1], F32)
        nc.scalar.activation(out=ET[:].rearrange("p r s f -> p (r s f)"),
                             in_=kallT[:].rearrange("p r s f -> p (r s f)"),
                             func=AF.Exp, accum_out=rs[:])
        pz = ps.tile([1, 1], F32, tag="setup")
        nc.tensor.matmul(pz[:], ones[:], rs[:], start=True, stop=True)
        z1 = small.tile([1, 1], F32)
        nc.vector.reciprocal(z1[:], pz[:])
        nc.vector.tensor_scalar_mul(z1[:], z1[:], 0.5 * ALPHA)
        scv = small.tile([P, 1], F32)
        nc.gpsimd.partition_broadcast(scv[:], z1[:])
        # KS[:, a, b, :] = T(ET[:, a, b, :]) + ET[:, b, a, :]  (unscaled E+E^T)
        KSb = big.tile([P, NB, NB, P], BF16, tag="KSb")
        KS = None
        if debug:
            KS = big.tile([P, NB, NB, P], F32, tag="KS")
            scn = small.tile([P, 1], F32)   # 0.5 * N / Z for debug K output
            zn = small.tile([1, 1], F32)
            nc.vector.reciprocal(zn[:], pz[:])
            nc.vector.tensor_scalar_mul(zn[:], zn[:], 0.5 * N)
            nc.gpsimd.partition_broadcast(scn[:], zn[:])
        for a in range(NB):
            for b in range(NB):
                pt = ps.tile([P, P], BF16, tag="setup" if b % 2 == 0 else "setup2")
                nc.tensor.transpose(pt[:], ET[:, a, b, :], sIDB[:])
                nc.vector.tensor_add(KSb[:, a, b, :], ET[:, b, a, :], pt[:])
                if debug:
                    nc.vector.tensor_add(KS[:, a, b, :], ET[:, b, a, :], pt[:])
                    nc.vector.tensor_scalar_mul(KS[:, a, b, :], KS[:, a, b, :],
                                                scn[:])
        if debug:
            nc.sync.dma_start(out=ksym_ext[:],
                              in_=KS[:].rearrange("p a b f -> p a (b f)"))

        # ---- ODE: explicit Euler, fully replicated ----
        traj = big.tile([P, steps, NB], F32, tag="traj")
        th0 = small.tile([P, NB], F32)
        nc.sync.dma_start(out=th0[:], in_=ins["theta0"][:])
        for t in range(steps):
            prev = th0[:] if t == 0 else traj[:, t - 1, :]
            wb = work.tile([P, NB, 3], F32, tag="wb")
            for k, shift in enumerate((PI / 2, 0.0, -PI / 2)):
                nc.vector.add_range_wrap(out=wb[:, :, k], in_=prev,
                                         shift=shift, bound=PI, period=2 * PI)
            # sall[:, a, :] = (cos, sin, -cos) of theta[a*128 + p]; the sin
            # channel is ACT-exact 0 at theta=0, so dtheta(0) == 0 exactly.
            sall = work.tile([P, NB, 3], BF16, tag="sall")
            nc.scalar.activation(out=sall[:], in_=wb[:], func=AF.Sin)
            po = ps.tile([P, NB, 2], F32, tag="setup")
            for ib in range(NB):
                for jb in range(NB):
                    nc.tensor.matmul(po[:, ib, :], KSb[:, jb, ib, :],
                                     sall[:, jb, 1:3], start=(jb == 0),
                                     stop=(jb == NB - 1))
            dd = work.tile([P, NB, 2], F32, tag="dd")
            nc.vector.tensor_mul(dd[:], po[:], sall[:, :, 0:2])
            d1 = work.tile([P, NB], F32, tag="d1")
            nc.vector.tensor_reduce(out=d1[:], in_=dd[:], axis=AX.X, op=OP.add)
            nc.vector.scalar_tensor_tensor(out=traj[:, t, :], in0=d1[:],
                                           scalar=scv[:], in1=prev,
                                           op0=OP.mult, op1=OP.add)
        nc.sync.dma_start(out=traj_ext[:], in_=traj[:])

    nc.compile()
    return nc


_CACHED = {}


def _get_program(steps=STEPS, debug=False):
    key = (steps, debug)
    if key not in _CACHED:
        _CACHED[key] = build_program(steps, debug)
    return _CACHED[key]


def make_in_maps(inputs, theta0=None):
    x = np.ascontiguousarray(np.asarray(inputs["x"], dtype=np.float32))
    w1 = np.asarray(inputs["w1"], np.float32)
    if theta0 is None:
        th0 = np.zeros((P, NB), np.float32)
    else:
        th0 = np.ascontiguousarray(
            np.asarray(theta0, np.float32).reshape(NB, P).T)
    base = {
        "xT": np.ascontiguousarray(x.T),
        "x8": np.ascontiguousarray(x.reshape(NB, P, FD)),
        "w1a": np.ascontiguousarray(w1[:FD]),
        "w1b": np.ascontiguousarray(w1[FD:]),
        "w2": np.asarray(inputs["w2"], np.float32),
        "w3l": np.asarray(inputs["w3"], np.float32).reshape(H, 1),
        "b1": np.asarray(inputs["b1"], np.float32).reshape(H, 1),
        "g1": np.asarray(inputs["gamma1"], np.float32).reshape(H, 1),
        "be1": np.asarray(inputs["beta1"], np.float32).reshape(H, 1),
        "g2": np.asarray(inputs["gamma2"], np.float32).reshape(H, 1),
        "be2": np.asarray(inputs["beta2"], np.float32).reshape(H, 1),
        "ident": np.eye(P, dtype=np.float32),
        "theta0": th0,
    }
    maps = []
    for c in range(NCORES):
        m = dict(base)
        m["xbT"] = np.ascontiguousarray(x[c * P:(c + 1) * P].T)
        maps.append(m)
    return maps


def unpack_traj(traj_pb, steps):
    return np.ascontiguousarray(
        traj_pb.transpose(1, 2, 0).reshape(steps, N).astype(np.float32))


def unpack_ksym(ksym_dbg):
    return np.ascontiguousarray(
        ksym_dbg.reshape(P, NB, N).transpose(1, 0, 2).reshape(N, N))


def run(inputs, steps=STEPS, theta0=None, debug=True):
    nc = _get_program(steps, debug)
    res = run_bass_kernel_spmd(nc, make_in_maps(inputs, theta0),
                               list(range(NCORES)))
    return res.results


def kernel(**inputs):
    results = run(inputs, debug=False)
    return unpack_traj(results[0]["traj_pb"], STEPS)
